# revision 7
# baseline (speedup 1.0000x reference)
"""Trainium2 Bass kernel for nn_PivNet (grid-hash KNN retrieval + 4-layer MLP).

Self-contained: hardcodes shapes/sharding for the graded problem.
Strategy: pure data parallel over 8 cores (65536 queries each); the
[knnd|pivots] table (bf16, 13MB) and MLP weights are replicated per core.

v5 (all-bf16):
 - fchunks of 1024 queries; MLP feature-major, N=512 matmuls writing
   512-wide halves of paired [128,1024] PSUM tiles, so each relu covers
   two m-blocks in one op.
 - b1 folded into W1 as a 106th constant-one feature row, so every relu
   is bias-free and pairable; b2/b3 are zeros for this problem (general
   unpaired fallback kept).
 - gathers issued 2 fchunks ahead (front_a) so SWDGE descriptor
   generation fully overlaps the MLP; gather-dependent distance math
   (front_b) runs after the MLP so the DVE FIFO never blocks on an
   in-flight gather.
 - fp8 was tried and is accuracy-dead (~5e-2 rel err vs 2e-2 budget).
Query/knnd normalization is folded into W1 on the host (f64).

t_bf per-j block layout (TW=112):
  [one(1) x(4) dist(1) knnd(100) piv(4) pad(2)]
transpose takes cols 0:106 -> featT rows [one, x, dist, knnd]; the
gather writes cols 6:110 ([knnd|piv] table row); pivots are excluded
from the transpose.
"""
from contextlib import ExitStack

import numpy as np

import concourse.bacc as bacc
import concourse.bass as bass
import concourse.tile as tile
from concourse import mybir
from concourse.masks import make_identity

NCORES = 8
B = 524288
DIM = 4
GRID = 16
K = 100
H = 512
FEAT = DIM + 1 + K      # 105
FEAT2 = FEAT + 1        # 106 (with folded-bias one-row)
TBL = GRID ** DIM       # 65536
P = 128
NQ = B // NCORES        # 65536
FCHUNK = 1024           # queries per fchunk
JT = FCHUNK // P        # 8 tiles of 128
TW = 112

F32 = mybir.dt.float32
BF16 = mybir.dt.bfloat16
I32 = mybir.dt.int32
AL = mybir.AluOpType
AF = mybir.ActivationFunctionType


def build_nc(mm_dt=BF16, reps=1, nq=NQ, inv_cd2=64.0, hw_loop=False,
             zero_b23=True):
    assert mm_dt == BF16, "v5 kernel is bf16-only"
    nf = nq // FCHUNK
    nc = bacc.Bacc("TRN2", target_bir_lowering=False, debug=False,
                   num_devices=NCORES)

    xq = nc.dram_tensor("xq", [nq, DIM], F32, kind="ExternalInput")
    tbl = nc.dram_tensor("tbl", [TBL, K + DIM], BF16, kind="ExternalInput")
    w1d = nc.dram_tensor("w1d", [FEAT2, H], BF16, kind="ExternalInput")
    w2d = nc.dram_tensor("w2d", [P, 4 * H], BF16, kind="ExternalInput")
    w3d = nc.dram_tensor("w3d", [P, 4 * H], BF16, kind="ExternalInput")
    w4d = nc.dram_tensor("w4d", [P, 4 * K], BF16, kind="ExternalInput")
    b2d = nc.dram_tensor("b2d", [P, 4], F32, kind="ExternalInput")
    b3d = nc.dram_tensor("b3d", [P, 4], F32, kind="ExternalInput")
    b4d = nc.dram_tensor("b4d", [P, 1], F32, kind="ExternalInput")
    cst_d = nc.dram_tensor("cst_d", [1, 104], F32, kind="ExternalInput")
    out_d = nc.dram_tensor("out", [nq, K], F32, kind="ExternalOutput")

    with tile.TileContext(nc) as tc:
        with ExitStack() as ctx:
            st = ctx.enter_context(tc.tile_pool(name="static", bufs=1))
            ident = st.tile([P, P], F32, tag="ident", name="ident")
            make_identity(nc, ident[:, :])
            identB = st.tile([P, P], BF16, tag="identB", name="identB")
            nc.vector.tensor_copy(out=identB[:, :], in_=ident[:, :])
            w1s = st.tile([FEAT2, H], BF16, tag="w1s", name="w1s")
            nc.sync.dma_start(out=w1s[:, :], in_=w1d[:, :])
            w2s = st.tile([P, 4 * H], BF16, tag="w2s", name="w2s")
            nc.sync.dma_start(out=w2s[:, :], in_=w2d[:, :])
            w3s = st.tile([P, 4 * H], BF16, tag="w3s", name="w3s")
            nc.sync.dma_start(out=w3s[:, :], in_=w3d[:, :])
            w4s = st.tile([P, 4 * K], BF16, tag="w4s", name="w4s")
            nc.sync.dma_start(out=w4s[:, :], in_=w4d[:, :])
            b2s = st.tile([P, 4], F32, tag="b2s", name="b2s")
            nc.sync.dma_start(out=b2s[:, :], in_=b2d[:, :])
            b3s = st.tile([P, 4], F32, tag="b3s", name="b3s")
            nc.sync.dma_start(out=b3s[:, :], in_=b3d[:, :])
            b4s = st.tile([P, 1], F32, tag="b4s", name="b4s")
            nc.sync.dma_start(out=b4s[:, :], in_=b4d[:, :])
            cst = st.tile([P, 104], F32, tag="cst", name="cst")
            nc.sync.dma_start(out=cst[:, :],
                              in_=cst_d[:, :].to_broadcast((P, 104)))

            xp = ctx.enter_context(tc.tile_pool(name="xp", bufs=3))
            ip = ctx.enter_context(tc.tile_pool(name="ip", bufs=3))
            tp = ctx.enter_context(tc.tile_pool(name="tp", bufs=2))
            fp = ctx.enter_context(tc.tile_pool(name="fp", bufs=2))
            hp = ctx.enter_context(tc.tile_pool(name="hp", bufs=2))
            op_ = ctx.enter_context(tc.tile_pool(name="op", bufs=2))
            psm_p = ctx.enter_context(
                tc.tile_pool(name="psm", bufs=2, space="PSUM"))
            pst_p = ctx.enter_context(
                tc.tile_pool(name="pst", bufs=2, space="PSUM"))
            pso_p = ctx.enter_context(
                tc.tile_pool(name="pso", bufs=2, space="PSUM"))

            def front_a(f):
                """x load, bin-index math, issue gathers into t_bf."""
                c0 = f * FCHUNK
                xt = xp.tile([P, 4 * JT], F32, tag="xt", name="xt")
                nc.sync.dma_start(
                    out=xt[:, :].rearrange("p (j d) -> p j d", j=JT),
                    in_=xq[c0:c0 + FCHUNK, :].rearrange(
                        "(j p) d -> p j d", p=P))
                xs = ip.tile([P, 4 * JT], F32, tag="xs", name="xs")
                nc.vector.tensor_tensor(
                    out=xs[:, :], in0=xt[:, :], in1=cst[:, 0:32],
                    op=AL.mult)
                xs2 = ip.tile([P, 4 * JT], F32, tag="xs2", name="xs2")
                nc.vector.tensor_tensor(
                    out=xs2[:, :], in0=xs[:, :], in1=cst[:, 32:64],
                    op=AL.add)
                vi = ip.tile([P, 4 * JT], I32, tag="vi", name="vi")
                nc.vector.tensor_copy(out=vi[:, :], in_=xs2[:, :])
                vf = ip.tile([P, 4 * JT], F32, tag="vf", name="vf")
                nc.vector.tensor_copy(out=vf[:, :], in_=vi[:, :])
                vg = ip.tile([P, 4 * JT], F32, tag="vg", name="vg")
                nc.vector.tensor_tensor(
                    out=vg[:, :], in0=vf[:, :], in1=xs2[:, :],
                    op=AL.is_gt)
                fl = ip.tile([P, 4 * JT], F32, tag="fl", name="fl")
                nc.vector.tensor_tensor(
                    out=fl[:, :], in0=vf[:, :], in1=vg[:, :],
                    op=AL.subtract)
                flc = ip.tile([P, 4 * JT], F32, tag="flc", name="flc")
                nc.vector.tensor_scalar(
                    out=flc[:, :], in0=fl[:, :],
                    scalar1=float(GRID - 1), scalar2=0.0,
                    op0=AL.min, op1=AL.max)
                rm = ip.tile([P, 4 * JT], F32, tag="rm", name="rm")
                nc.vector.tensor_tensor(
                    out=rm[:, :], in0=flc[:, :], in1=cst[:, 64:96],
                    op=AL.mult)
                rmv = rm[:, :].rearrange("p (a b) -> p a b", b=2)
                r1 = ip.tile([P, 2 * JT], F32, tag="r1", name="r1")
                nc.vector.tensor_tensor(
                    out=r1[:, :], in0=rmv[:, :, 0], in1=rmv[:, :, 1],
                    op=AL.add)
                r1v = r1[:, :].rearrange("p (a b) -> p a b", b=2)
                idx4 = ip.tile([P, JT], I32, tag="idx4", name="idx4")
                nc.vector.tensor_tensor(
                    out=idx4[:, :], in0=r1v[:, :, 0], in1=r1v[:, :, 1],
                    op=AL.add)

                t_bf = tp.tile([P, JT * TW], BF16, tag="tbf", name="tbf")
                tv = t_bf[:, :].rearrange("p (j c) -> p j c", j=JT)
                nc.vector.tensor_copy(
                    out=tv[:, :, 0:1],
                    in_=cst[:, 96:104].rearrange("p (j o) -> p j o", o=1))
                nc.vector.tensor_copy(
                    out=tv[:, :, 1:1 + DIM],
                    in_=xt[:, :].rearrange("p (j d) -> p j d", j=JT))
                for j in range(JT):
                    nc.gpsimd.indirect_dma_start(
                        out=t_bf[:, j * TW + 6:j * TW + 110],
                        out_offset=None,
                        in_=tbl[:, :],
                        in_offset=bass.IndirectOffsetOnAxis(
                            ap=idx4[:, j:j + 1], axis=0))
                return xt, t_bf

            def front_b(xt, t_bf):
                """dist math (needs gathers done)."""
                tv = t_bf[:, :].rearrange("p (j c) -> p j c", j=JT)
                dx = ip.tile([P, 4 * JT], F32, tag="dx", name="dx")
                nc.vector.tensor_tensor(
                    out=dx[:, :].rearrange("p (j d) -> p j d", j=JT),
                    in0=tv[:, :, 106:110],
                    in1=xt[:, :].rearrange("p (j d) -> p j d", j=JT),
                    op=AL.subtract)
                sq = ip.tile([P, 4 * JT], F32, tag="sq", name="sq")
                nc.vector.tensor_tensor(
                    out=sq[:, :], in0=dx[:, :], in1=dx[:, :],
                    op=AL.mult)
                sqv = sq[:, :].rearrange("p (a b) -> p a b", b=2)
                q1 = ip.tile([P, 2 * JT], F32, tag="q1", name="q1")
                nc.vector.tensor_tensor(
                    out=q1[:, :], in0=sqv[:, :, 0], in1=sqv[:, :, 1],
                    op=AL.add)
                q1v = q1[:, :].rearrange("p (a b) -> p a b", b=2)
                d2a = ip.tile([P, JT], F32, tag="d2a", name="d2a")
                nc.vector.tensor_tensor(
                    out=d2a[:, :], in0=q1v[:, :, 0], in1=q1v[:, :, 1],
                    op=AL.add)
                nc.scalar.activation(
                    out=tv[:, :, 5:6], in_=d2a[:, :].rearrange(
                        "p (j o) -> p j o", o=1),
                    func=AF.Sqrt, scale=float(inv_cd2))

            def trans(t_bf, featT):
                """8 PE transposes -> featT [106, 1024] bf16."""
                for hh in range(2):
                    pst = pst_p.tile([FEAT2, 512], BF16, tag="pst",
                                     name="pst")
                    for jj in range(4):
                        j = hh * 4 + jj
                        nc.tensor.transpose(
                            out=pst[:, jj * P:(jj + 1) * P],
                            in_=t_bf[:, j * TW:j * TW + FEAT2],
                            identity=identB[:, :])
                    nc.vector.tensor_copy(
                        out=featT[:, hh * 512:(hh + 1) * 512],
                        in_=pst[:, :])
                return featT

            def relu_pair(eng, out, ps):
                if eng == "act":
                    nc.scalar.activation(out=out, in_=ps, func=AF.Relu)
                else:
                    nc.vector.tensor_scalar(
                        out=out, in0=ps, scalar1=0.0, scalar2=None,
                        op0=AL.max)

            def relu_bias(out, ps, bias):
                nc.scalar.activation(out=out, in_=ps, func=AF.Relu,
                                     bias=bias)

            def layer(hin, hout, ws, bs, engs, lhsT_of_m):
                """one hidden layer for 512 queries: 2 psum pairs."""
                for pair in range(2):
                    ps = psm_p.tile([P, 1024], F32, tag="psm", name="psm")
                    for mi in range(2):
                        m = pair * 2 + mi
                        for k in range(4):
                            lt, st_, sp = lhsT_of_m(m, k)
                            nc.tensor.matmul(
                                out=ps[:, mi * 512:(mi + 1) * 512],
                                lhsT=lt, rhs=hin[:, k * H:(k + 1) * H],
                                start=st_, stop=sp)
                    if zero_b23 or bs is None:
                        relu_pair(engs[pair], hout[:, pair * 1024:
                                                   (pair + 1) * 1024],
                                  ps[:, :])
                    else:
                        for mi in range(2):
                            m = pair * 2 + mi
                            relu_bias(hout[:, m * H:(m + 1) * H],
                                      ps[:, mi * 512:(mi + 1) * 512],
                                      bs[:, m:m + 1])

            def mlp_half(f, h, featT):
                c0 = f * FCHUNK + h * 512
                fv = featT[:, h * 512:(h + 1) * 512]
                h1 = hp.tile([P, 4 * H], BF16, tag="h1", name="h1")
                for pair in range(2):
                    ps = psm_p.tile([P, 1024], F32, tag="psm", name="psm")
                    for mi in range(2):
                        m = pair * 2 + mi
                        nc.tensor.matmul(
                            out=ps[:, mi * 512:(mi + 1) * 512],
                            lhsT=w1s[:, m * P:(m + 1) * P],
                            rhs=fv, start=True, stop=True)
                    relu_pair(("act", "dve")[pair],
                              h1[:, pair * 1024:(pair + 1) * 1024],
                              ps[:, :])
                h2 = hp.tile([P, 4 * H], BF16, tag="h2", name="h2")
                layer(h1, h2, w2s, b2s, ("act", "dve"),
                      lambda m, k: (w2s[:, k * H + m * P:k * H + m * P + P],
                                    k == 0, k == 3))
                h3 = hp.tile([P, 4 * H], BF16, tag="h3", name="h3")
                layer(h2, h3, w3s, b3s, ("act", "act"),
                      lambda m, k: (w3s[:, k * H + m * P:k * H + m * P + P],
                                    k == 0, k == 3))
                p4 = psm_p.tile([P, 1024], F32, tag="psm", name="psm")
                for k in range(4):
                    nc.tensor.matmul(
                        out=p4[0:K, 0:512],
                        lhsT=w4s[:, k * K:(k + 1) * K],
                        rhs=h3[:, k * H:(k + 1) * H],
                        start=(k == 0), stop=(k == 3))
                o4 = hp.tile([P, H], BF16, tag="o4", name="o4")
                nc.vector.tensor_scalar(
                    out=o4[0:K, :], in0=p4[0:K, 0:512],
                    scalar1=b4s[0:K, 0:1], scalar2=None, op0=AL.add)
                for g in range(2):
                    po = pso_p.tile([P, 2 * K], BF16, tag="pso",
                                    name="pso")
                    for jj in range(2):
                        nc.tensor.transpose(
                            out=po[:, jj * K:(jj + 1) * K],
                            in_=o4[0:K, (2 * g + jj) * P:
                                   (2 * g + jj + 1) * P],
                            identity=identB[0:K, 0:K])
                    ot = op_.tile([P, 2 * K], F32, tag=f"ot{g}",
                                  name="ot")
                    nc.vector.tensor_copy(out=ot[:, :], in_=po[:, :])
                    nc.sync.dma_start(
                        out=out_d[c0 + 2 * g * P:c0 + 2 * (g + 1) * P, :]
                        .rearrange("(j p) k -> p j k", p=P),
                        in_=ot[:, :].rearrange("p (j k) -> p j k", j=2))

            if hw_loop:
                loop_cm = tc.For_i(0, reps, name="reploop")
                loop_cm.__enter__()
                py_reps = 1
            else:
                py_reps = reps
            for _ in range(py_reps):
                fr = {0: front_a(0)}
                if nf > 1:
                    fr[1] = front_a(1)
                front_b(*fr[0])
                fT0 = fp.tile([FEAT2, FCHUNK], BF16, tag="featT",
                              name="featT")
                featTs = {0: trans(fr[0][1], fT0)}
                for f in range(nf):
                    if f + 2 < nf:
                        fr[f + 2] = front_a(f + 2)
                    featT = featTs.pop(f)
                    mlp_half(f, 0, featT)
                    mlp_half(f, 1, featT)
                    fr.pop(f)
                    if f + 1 < nf:
                        front_b(*fr[f + 1])
                        fT = fp.tile([FEAT2, FCHUNK], BF16, tag="featT",
                                     name="featT")
                        featTs[f + 1] = trans(fr[f + 1][1], fT)
            if hw_loop:
                loop_cm.__exit__(None, None, None)
    nc.finalize()
    return nc


def prep_in_maps(inputs, mm_np=None, nq=NQ):
    """Host-side prep: fold normalization + b1 into W1, pack weights.
    Returns (in_maps list for 8 cores, inv_cd2 float, zero_b23 bool)."""
    import ml_dtypes
    bf16 = ml_dtypes.bfloat16
    f64 = np.float64
    x = np.ascontiguousarray(np.asarray(inputs["x"], np.float32))
    mins = np.asarray(inputs["min_values"], f64)
    maxs = np.asarray(inputs["max_values"], f64)
    pivots = np.asarray(inputs["pivots"], np.float32)
    knnd = np.asarray(inputs["knnd"], np.float32)
    qm = np.asarray(inputs["query_mean"], f64)
    qs = np.asarray(inputs["query_std"], f64)
    km = np.asarray(inputs["knnd_mean"], f64)
    ks = np.asarray(inputs["knnd_std"], f64)
    W1 = np.asarray(inputs["W1"], f64)
    b1 = np.asarray(inputs["b1"], f64)
    W2 = np.asarray(inputs["W2"], np.float32)
    b2 = np.asarray(inputs["b2"], np.float32)
    W3 = np.asarray(inputs["W3"], np.float32)
    b3 = np.asarray(inputs["b3"], np.float32)
    W4 = np.asarray(inputs["W4"], np.float32)
    b4 = np.asarray(inputs["b4"], np.float32)

    table = np.ascontiguousarray(
        np.concatenate([knnd, pivots], axis=1).astype(bf16))  # [65536,104]

    s_vec = np.concatenate([qs, [1.0], ks])               # [105]
    m_vec = np.concatenate([qm, [0.0], km])               # [105]
    W1n = W1 / s_vec[:, None]                             # [105, 512]
    b1p = b1 - (m_vec / s_vec) @ W1                       # [512]
    # row order: [one(=b1), x(4), dist(1), knnd(100)]
    W1f = np.concatenate([b1p[None, :], W1n], axis=0)     # [106, 512]
    W1p = W1f.astype(bf16)

    rng = maxs - mins
    sc = (GRID / rng).astype(np.float32)                  # [4]
    sh = (-mins * (GRID / rng)).astype(np.float32)        # [4]
    inv_cd2 = float(1.0 / np.sum((rng / GRID) ** 2))
    cst = np.zeros((1, 104), np.float32)
    cst[0, 0:32] = np.tile(sc, JT)
    cst[0, 32:64] = np.tile(sh, JT)
    radix = np.array([GRID ** 3, GRID ** 2, GRID, 1], np.float32)
    cst[0, 64:96] = np.tile(radix, JT)
    cst[0, 96:104] = 1.0

    w2p = np.ascontiguousarray(
        W2.reshape(4, P, H).transpose(1, 0, 2).reshape(P, 4 * H)).astype(bf16)
    w3p = np.ascontiguousarray(
        W3.reshape(4, P, H).transpose(1, 0, 2).reshape(P, 4 * H)).astype(bf16)
    w4p = np.ascontiguousarray(
        W4.reshape(4, P, K).transpose(1, 0, 2).reshape(P, 4 * K)).astype(bf16)
    b2m = np.ascontiguousarray(b2.reshape(4, P).T)
    b3m = np.ascontiguousarray(b3.reshape(4, P).T)
    b4m = np.zeros((P, 1), np.float32)
    b4m[0:K, 0] = b4

    shared = dict(tbl=table, w1d=W1p, w2d=w2p, w3d=w3p, w4d=w4p,
                  b2d=b2m, b3d=b3m, b4d=b4m, cst_d=cst)
    in_maps = [dict(shared, xq=x[c * nq:(c + 1) * nq]) for c in range(NCORES)]
    zero_b23 = bool(np.all(b2 == 0) and np.all(b3 == 0))
    return in_maps, inv_cd2, zero_b23


def kernel(**inputs):
    from concourse.bass_utils import run_bass_kernel_spmd
    in_maps, inv_cd2, zero_b23 = prep_in_maps(inputs)
    nc = build_nc(mm_dt=BF16, reps=1, inv_cd2=inv_cd2, zero_b23=zero_b23)
    res = run_bass_kernel_spmd(nc, in_maps, list(range(NCORES)))
    out = np.concatenate(
        [np.asarray(res.results[c]["out"]) for c in range(NCORES)], axis=0)
    return out.astype(np.float32)


if __name__ == "__main__":
    rng = np.random.default_rng(0)
    fake = {
        "x": rng.random((B, DIM)).astype(np.float32),
        "min_values": np.zeros(DIM, np.float32),
        "max_values": np.ones(DIM, np.float32),
        "pivots": rng.random((TBL, DIM)).astype(np.float32),
        "knnd": rng.random((TBL, K)).astype(np.float32),
        "query_mean": rng.standard_normal(DIM).astype(np.float32),
        "query_std": (0.5 + rng.random(DIM)).astype(np.float32),
        "knnd_mean": rng.standard_normal(K).astype(np.float32),
        "knnd_std": (0.5 + rng.random(K)).astype(np.float32),
        "W1": (0.05 * rng.standard_normal((FEAT, H))).astype(np.float32),
        "b1": np.zeros(H, np.float32),
        "W2": (0.05 * rng.standard_normal((H, H))).astype(np.float32),
        "b2": np.zeros(H, np.float32),
        "W3": (0.05 * rng.standard_normal((H, H))).astype(np.float32),
        "b3": np.zeros(H, np.float32),
        "W4": (0.05 * rng.standard_normal((H, K))).astype(np.float32),
        "b4": np.zeros(K, np.float32),
    }
    o = kernel(**fake)
    print("out", o.shape, o.dtype, float(np.abs(o).mean()))
